# revision 11
# baseline (speedup 1.0000x reference)
"""Class-balanced softmax cross-entropy loss on 8 Trainium2 NeuronCores.

Math: counts N_c over batch; w_c = (1-beta)/(1-beta^N_c) (0 if N_c=0);
loss = -sum w[t](logp[t]) / sum w[t] over valid pixels.

Fast path (used when all class weights are equal, which holds whenever every
class count N_c is large enough that beta^N_c underflows — always true for
this problem's 4.2M uniformly distributed pixels; verified exactly on host
via bincount): the weights cancel in the ratio, so
  loss = (sum_pix lse - sum_pix x[t]) / N_valid
Device computes per core: G1 = sum x[t] (gather via 19 is_equal stt accums
per chunk), G2 = sum_{t>=0} lse (one masked stt per chunk), with
lse = ln(sum_c exp(x_c)) via ACT exp + DVE pairwise-tree adds (f16 TT ops
hit the 2x DVE mode; strided tensor_reduce does not).
Inputs are host-cast: logits -> bf16 (halves HBM traffic; error ~1e-3 rel),
target -> f32.

Exact fallback path (any weight spread): original per-class A/B/N kernel.
"""

import numpy as np
import sys

for _p in ("/opt/trn_rl_repo",):
    if _p not in sys.path:
        sys.path.insert(0, _p)

import ml_dtypes
from concourse import bass, mybir
from concourse.bass_utils import run_bass_kernel_spmd

NCLASS = 19
BETA = 0.999
NCORES = 8
P = 128
COLS = 4096              # 512*1024 / 128
F = 1024                 # free-dim chunk
NCHUNK = COLS // F       # 4

f32 = mybir.dt.float32
f16 = mybir.dt.float16
bf16 = mybir.dt.bfloat16
i32 = mybir.dt.int32
AF = mybir.ActivationFunctionType
ALU = mybir.AluOpType

NG = NCHUNK * NCLASS         # 76 gather columns
ACC_COLS = NG + NCHUNK       # + per-chunk masked-lse columns = 80


def _build_fast():
    nc = bass.Bass()
    logits = nc.declare_dram_parameter("logits", [NCLASS, P, COLS], bf16, isOutput=False)
    target = nc.declare_dram_parameter("target", [P, COLS], f32, isOutput=False)
    out = nc.declare_dram_parameter("out", [1, ACC_COLS], f32, isOutput=True)

    EF = NCLASS * F
    X2 = nc.alloc_sbuf_tensor("X2", [P, 2 * EF], bf16)
    E2 = nc.alloc_sbuf_tensor("E2", [P, 2 * EF], f16)
    T = nc.alloc_sbuf_tensor("T", [P, COLS], f32)
    L2 = nc.alloc_sbuf_tensor("L2", [P, 2 * F], f16)
    junk = nc.alloc_sbuf_tensor("junk", [P, F], f16)
    ABN = nc.alloc_sbuf_tensor("ABN", [P, ACC_COLS], f32)
    ones = nc.alloc_sbuf_tensor("ones", [P, 1], f32)
    res = nc.alloc_sbuf_tensor("res", [1, ACC_COLS], f32)
    ps = nc.alloc_psum_tensor("ps", [1, ACC_COLS], f32)

    with (
        nc.Block() as block,
        nc.semaphore("s_x") as s_x,
        nc.semaphore("s_t") as s_t,
        nc.semaphore("s_exp") as s_exp,
        nc.semaphore("s_tree") as s_tree,
        nc.semaphore("s_log") as s_log,
        nc.semaphore("s_gA") as s_gA,
        nc.semaphore("s_gB") as s_gB,
        nc.semaphore("s_mm") as s_mm,
        nc.semaphore("s_out") as s_out,
        nc.allow_low_precision("f16 tree-sum of exp; error ~0.1% on lse"),
    ):
        @block.sync
        def _(sp):
            sp.dma_start(T[:], target[:, :]).then_inc(s_t, 16)
            for k in range(NCHUNK):
                h = k % 2
                if k >= 2:
                    # X half reused: chunk k-2's gathers (which read X) done
                    sp.wait_ge(s_gA, k - 1)
                sp.dma_start(
                    X2[:, h * EF:(h + 1) * EF].rearrange("p (c f) -> p c f", c=NCLASS),
                    logits[:, :, k * F:(k + 1) * F].rearrange("c p f -> p c f"),
                ).then_inc(s_x, 16)

        @block.scalar
        def _(act):
            for k in range(NCHUNK):
                h = k % 2
                act.wait_ge(s_x, 16 * (k + 1))
                E = E2[:, h * EF:(h + 1) * EF]
                X = X2[:, h * EF:(h + 1) * EF]
                for c in range(NCLASS):
                    ins = act.activation(
                        E[:, c * F:(c + 1) * F], X[:, c * F:(c + 1) * F], AF.Exp)
                    if c == NCLASS - 1:
                        ins.then_inc(s_exp, 1)
                # tree folds sumexp into E[:, 0:F]
                act.wait_ge(s_tree, k + 1)
                if k >= 2:
                    act.wait_ge(s_gB, k - 1)   # L half reused
                act.activation(
                    L2[:, h * F:(h + 1) * F], E[:, 0:F], AF.Ln,
                ).then_inc(s_log, 1)
            # tail: psum -> sbuf -> dram
            act.wait_ge(s_mm, 1)
            act.copy(res[:], ps[:])
            act.dma_start(out[:, :], res[:]).then_inc(s_out, 16)
            act.wait_ge(s_out, 16)

        @block.vector
        def _(dve):
            dve.memset(ABN[:], 0.0)
            dve.memset(ones[:], 1.0)
            dve.wait_ge(s_t, 16)
            for k in range(NCHUNK):
                h = k % 2
                dve.wait_ge(s_exp, k + 1)   # E ready (implies X landed)
                E = E2[:, h * EF:(h + 1) * EF]
                # pairwise tree: 19 -> 16 -> 8 -> 4 -> 2 -> 1 (contiguous f16
                # TT adds run in the DVE 2x mode)
                dve.tensor_tensor(out=E[:, 0:3 * F], in0=E[:, 0:3 * F],
                                  in1=E[:, 16 * F:19 * F], op=ALU.add)
                dve.tensor_tensor(out=E[:, 0:8 * F], in0=E[:, 0:8 * F],
                                  in1=E[:, 8 * F:16 * F], op=ALU.add)
                dve.tensor_tensor(out=E[:, 0:4 * F], in0=E[:, 0:4 * F],
                                  in1=E[:, 4 * F:8 * F], op=ALU.add)
                dve.tensor_tensor(out=E[:, 0:2 * F], in0=E[:, 0:2 * F],
                                  in1=E[:, 2 * F:4 * F], op=ALU.add)
                dve.tensor_tensor(out=E[:, 0:F], in0=E[:, 0:F],
                                  in1=E[:, F:2 * F], op=ALU.add).then_inc(s_tree, 1)
                # gather: G1 partials, one stt per class
                X = X2[:, h * EF:(h + 1) * EF]
                Tk = T[:, k * F:(k + 1) * F]
                for c in range(NCLASS):
                    ins = dve.scalar_tensor_tensor(
                        out=junk[:], in0=Tk, scalar=float(c),
                        in1=X[:, c * F:(c + 1) * F],
                        op0=ALU.is_equal, op1=ALU.mult,
                        accum_out=ABN[:, k * NCLASS + c: k * NCLASS + c + 1])
                    if c == NCLASS - 1:
                        ins.then_inc(s_gA, 1)
                # masked lse sum: G2 partial
                dve.wait_ge(s_log, k + 1)
                dve.scalar_tensor_tensor(
                    out=junk[:], in0=Tk, scalar=-0.5,
                    in1=L2[:, h * F:(h + 1) * F],
                    op0=ALU.is_gt, op1=ALU.mult,
                    accum_out=ABN[:, NG + k: NG + k + 1]).then_inc(s_gB, 1)

        @block.tensor
        def _(pe):
            pe.wait_ge(s_gB, NCHUNK)
            pe.matmul(ps[:], lhsT=ones[:], rhs=ABN[:], start=True, stop=True
                      ).then_inc(s_mm, 1)

    return nc


def _build_fast2():
    """No-ignore fast path: gathers reordered ahead of the exp-dependent
    tree, X DMAs split by class halves for a short pipeline fill, and the
    lse sum folded into the Ln op's accum_out on ACT."""
    nc = bass.Bass()
    logits = nc.declare_dram_parameter("logits", [NCLASS, P, COLS], bf16, isOutput=False)
    target = nc.declare_dram_parameter("target", [P, COLS], f32, isOutput=False)
    out = nc.declare_dram_parameter("out", [1, ACC_COLS], f32, isOutput=True)

    EF = NCLASS * F
    CSPLIT = 10              # classes 0..9 in the first DMA half
    X2 = nc.alloc_sbuf_tensor("X2", [P, 2 * EF], bf16)
    E2 = nc.alloc_sbuf_tensor("E2", [P, 2 * EF], f16)
    T = nc.alloc_sbuf_tensor("T", [P, COLS], f32)
    junk = nc.alloc_sbuf_tensor("junk", [P, F], f16)
    junkA = nc.alloc_sbuf_tensor("junkA", [P, F], f16)
    ABN = nc.alloc_sbuf_tensor("ABN", [P, ACC_COLS], f32)
    ones = nc.alloc_sbuf_tensor("ones", [P, 1], f32)
    res = nc.alloc_sbuf_tensor("res", [1, ACC_COLS], f32)
    ps = nc.alloc_psum_tensor("ps", [1, ACC_COLS], f32)

    with (
        nc.Block() as block,
        nc.semaphore("s_x") as s_x,
        nc.semaphore("s_t") as s_t,
        nc.semaphore("s_exp") as s_exp,
        nc.semaphore("s_tree") as s_tree,
        nc.semaphore("s_gA") as s_gA,
        nc.semaphore("s_lse") as s_lse,
        nc.semaphore("s_mm") as s_mm,
        nc.semaphore("s_out") as s_out,
        nc.allow_low_precision("f16 tree-sum of exp; error ~0.1% on lse"),
    ):
        @block.sync
        def _(sp):
            for k in range(NCHUNK):
                h = k % 2
                if k >= 2:
                    sp.wait_ge(s_gA, k - 1)
                    sp.wait_ge(s_exp, k - 1)
                sp.dma_start(T[:, k * F:(k + 1) * F],
                             target[:, k * F:(k + 1) * F]).then_inc(s_t, 16)
                Xh = X2[:, h * EF:(h + 1) * EF].rearrange("p (c f) -> p c f", c=NCLASS)
                src = logits[:, :, k * F:(k + 1) * F].rearrange("c p f -> p c f")
                if k == 0:
                    # fine-grained first chunk: four 5-class DMAs so compute
                    # starts as early as possible
                    for lo, hi in ((0, 5), (5, 10), (10, 15), (15, 19)):
                        sp.dma_start(Xh[:, lo:hi], src[:, lo:hi]).then_inc(s_x, 16)
                else:
                    sp.dma_start(Xh[:, :CSPLIT], src[:, :CSPLIT]).then_inc(s_x, 16)
                    sp.dma_start(Xh[:, CSPLIT:], src[:, CSPLIT:]).then_inc(s_x, 16)

        @block.scalar
        def _(act):
            for k in range(NCHUNK):
                h = k % 2
                E = E2[:, h * EF:(h + 1) * EF]
                X = X2[:, h * EF:(h + 1) * EF]
                if k == 0:
                    groups = [(0, 5, 16), (5, 10, 32), (10, 15, 48), (15, 19, 64)]
                else:
                    base = 64 + 32 * (k - 1)
                    groups = [(0, CSPLIT, base + 16), (CSPLIT, NCLASS, base + 32)]
                for lo, hi, thr in groups:
                    act.wait_ge(s_x, thr)
                    for c in range(lo, hi):
                        ins = act.activation(
                            E[:, c * F:(c + 1) * F], X[:, c * F:(c + 1) * F], AF.Exp)
                        if c == NCLASS - 1:
                            ins.then_inc(s_exp, 1)
                act.wait_ge(s_tree, k + 1)
                act.activation(
                    junkA[:], E[:, 0:F], AF.Ln,
                    accum_out=ABN[:, NG + k: NG + k + 1]).then_inc(s_lse, 1)
            act.wait_ge(s_mm, NCHUNK + 1)
            act.copy(res[:], ps[:])
            act.dma_start(out[:, :], res[:]).then_inc(s_out, 16)
            act.wait_ge(s_out, 16)

        @block.vector
        def _(dve):
            dve.memset(ABN[:], 0.0)
            dve.memset(ones[:], 1.0)
            for k in range(NCHUNK):
                h = k % 2
                X = X2[:, h * EF:(h + 1) * EF]
                E = E2[:, h * EF:(h + 1) * EF]
                Tk = T[:, k * F:(k + 1) * F]
                dve.wait_ge(s_t, 16 * (k + 1))

                def _tree():
                    dve.wait_ge(s_exp, k + 1)
                    dve.tensor_tensor(out=E[:, 0:3 * F], in0=E[:, 0:3 * F],
                                      in1=E[:, 16 * F:19 * F], op=ALU.add)
                    dve.tensor_tensor(out=E[:, 0:8 * F], in0=E[:, 0:8 * F],
                                      in1=E[:, 8 * F:16 * F], op=ALU.add)
                    dve.tensor_tensor(out=E[:, 0:4 * F], in0=E[:, 0:4 * F],
                                      in1=E[:, 4 * F:8 * F], op=ALU.add)
                    dve.tensor_tensor(out=E[:, 0:2 * F], in0=E[:, 0:2 * F],
                                      in1=E[:, 2 * F:4 * F], op=ALU.add)
                    dve.tensor_tensor(out=E[:, 0:F], in0=E[:, 0:F],
                                      in1=E[:, F:2 * F], op=ALU.add
                                      ).then_inc(s_tree, 1)

                def _gathers():
                    # descending class order opposes ACT's ascending exp
                    # order, minimizing same-region SBUF port contention
                    if k == 0:
                        groups = [(0, 5, 16), (5, 10, 32), (10, 15, 48), (15, 19, 64)]
                    else:
                        base = 64 + 32 * (k - 1)
                        groups = [(0, CSPLIT, base + 16),
                                  (CSPLIT, NCLASS, base + 32)]
                    for lo, hi, thr in groups:
                        dve.wait_ge(s_x, thr)
                        for c in reversed(range(lo, hi)):
                            ins = dve.scalar_tensor_tensor(
                                out=junk[:], in0=Tk, scalar=float(c),
                                in1=X[:, c * F:(c + 1) * F],
                                op0=ALU.is_equal, op1=ALU.mult,
                                accum_out=ABN[:, k * NCLASS + c: k * NCLASS + c + 1])
                            if c == lo and hi == NCLASS:
                                ins.then_inc(s_gA, 1)

                if k == NCHUNK - 1:
                    # last chunk: tree first so the ln/matmul tail overlaps
                    # the remaining gathers
                    _tree()
                    _gathers()
                else:
                    _gathers()
                    _tree()

        @block.tensor
        def _(pe):
            # per-chunk partition-reduces keep the tail short
            for k in range(NCHUNK):
                pe.wait_ge(s_gA, k + 1)
                pe.matmul(ps[:, k * NCLASS:(k + 1) * NCLASS], lhsT=ones[:],
                          rhs=ABN[:, k * NCLASS:(k + 1) * NCLASS],
                          start=True, stop=True).then_inc(s_mm, 1)
            pe.wait_ge(s_lse, NCHUNK)
            pe.matmul(ps[:, NG:], lhsT=ones[:], rhs=ABN[:, NG:],
                      start=True, stop=True).then_inc(s_mm, 1)

    return nc


def _build_exact():
    """Original per-class A/B/N kernel (correct for any weight pattern)."""
    nc = bass.Bass()
    F0 = 512
    NCH0 = COLS // F0
    SEC = NCH0 * NCLASS
    ACC0 = 3 * SEC
    logits = nc.declare_dram_parameter("logits", [NCLASS, P, COLS], f32, isOutput=False)
    target = nc.declare_dram_parameter("target", [P, COLS], i32, isOutput=False)
    out = nc.declare_dram_parameter("out", [1, ACC0], f32, isOutput=True)

    EF = NCLASS * F0
    X2 = nc.alloc_sbuf_tensor("X2", [P, 2 * EF], f32)
    E2 = nc.alloc_sbuf_tensor("E2", [P, 2 * EF], f32)
    Ti2 = nc.alloc_sbuf_tensor("Ti2", [P, 2 * F0], i32)
    Tf2 = nc.alloc_sbuf_tensor("Tf2", [P, 2 * F0], f32)
    S2 = nc.alloc_sbuf_tensor("S2", [P, 2 * F0], f32)
    L2 = nc.alloc_sbuf_tensor("L2", [P, 2 * F0], f32)
    junk = nc.alloc_sbuf_tensor("junk", [P, F0], f32)
    ABN = nc.alloc_sbuf_tensor("ABN", [P, ACC0], f32)
    ones = nc.alloc_sbuf_tensor("ones", [P, 1], f32)
    res = nc.alloc_sbuf_tensor("res", [1, ACC0], f32)
    ps = nc.alloc_psum_tensor("ps", [1, ACC0], f32)

    with (
        nc.Block() as block,
        nc.semaphore("sem_x") as sem_x,
        nc.semaphore("sem_t") as sem_t,
        nc.semaphore("sem_exp") as sem_exp,
        nc.semaphore("sem_red") as sem_red,
        nc.semaphore("sem_log") as sem_log,
        nc.semaphore("sem_done") as sem_done,
        nc.semaphore("sem_mm") as sem_mm,
        nc.semaphore("sem_out") as sem_out,
    ):
        @block.scalar
        def _(act):
            for k in range(NCH0):
                h = k % 2
                if k >= 2:
                    act.wait_ge(sem_done, k - 1)
                act.dma_start(
                    X2[:, h * EF:(h + 1) * EF].rearrange("p (c f) -> p c f", c=NCLASS),
                    logits[:, :, k * F0:(k + 1) * F0].rearrange("c p f -> p c f"),
                ).then_inc(sem_x, 16)
                act.dma_start(
                    Ti2[:, h * F0:(h + 1) * F0], target[:, k * F0:(k + 1) * F0],
                ).then_inc(sem_t, 16)
                act.wait_ge(sem_x, 16 * (k + 1))
                for c in range(NCLASS):
                    ins = act.activation(
                        E2[:, h * EF + c * F0: h * EF + (c + 1) * F0],
                        X2[:, h * EF + c * F0: h * EF + (c + 1) * F0], AF.Exp)
                    if c == NCLASS - 1:
                        ins.then_inc(sem_exp, 1)
                act.wait_ge(sem_red, k + 1)
                act.activation(
                    L2[:, h * F0:(h + 1) * F0], S2[:, h * F0:(h + 1) * F0], AF.Ln,
                ).then_inc(sem_log, 1)
            act.wait_ge(sem_mm, 1)
            act.copy(res[:], ps[:])
            act.dma_start(out[:, :], res[:]).then_inc(sem_out, 16)
            act.wait_ge(sem_out, 16)

        @block.vector
        def _(dve):
            dve.memset(ABN[:], 0.0)
            dve.memset(ones[:], 1.0)
            for k in range(NCH0):
                h = k % 2
                dve.wait_ge(sem_exp, k + 1)
                dve.tensor_reduce(
                    S2[:, h * F0:(h + 1) * F0],
                    E2[:, h * EF:(h + 1) * EF].rearrange("p (c f) -> p f c", c=NCLASS),
                    axis=mybir.AxisListType.X, op=ALU.add,
                ).then_inc(sem_red, 1)
                dve.wait_ge(sem_t, 16 * (k + 1))
                Ti = Tf2[:, h * F0:(h + 1) * F0]
                dve.tensor_copy(Ti[:], Ti2[:, h * F0:(h + 1) * F0])
                for c in range(NCLASS):
                    dve.scalar_tensor_tensor(
                        out=junk[:], in0=Ti[:], scalar=float(c),
                        in1=X2[:, h * EF + c * F0: h * EF + (c + 1) * F0],
                        op0=ALU.is_equal, op1=ALU.mult,
                        accum_out=ABN[:, 0 * SEC + k * NCLASS + c: 0 * SEC + k * NCLASS + c + 1])
                dve.wait_ge(sem_log, k + 1)
                LSE = L2[:, h * F0:(h + 1) * F0]
                for c in range(NCLASS):
                    dve.scalar_tensor_tensor(
                        out=junk[:], in0=Ti[:], scalar=float(c), in1=LSE[:],
                        op0=ALU.is_equal, op1=ALU.mult,
                        accum_out=ABN[:, 1 * SEC + k * NCLASS + c: 1 * SEC + k * NCLASS + c + 1])
                for c in range(NCLASS):
                    ins = dve.tensor_scalar(
                        out=junk[:], in0=Ti[:], scalar1=float(c), scalar2=None,
                        op0=ALU.is_equal, op1=ALU.add,
                        accum_out=ABN[:, 2 * SEC + k * NCLASS + c: 2 * SEC + k * NCLASS + c + 1])
                    if c == NCLASS - 1:
                        ins.then_inc(sem_done, 1)

        @block.tensor
        def _(pe):
            pe.wait_ge(sem_done, NCH0)
            pe.matmul(ps[:], lhsT=ones[:], rhs=ABN[:], start=True, stop=True).then_inc(sem_mm, 1)

    return nc


_CACHE = {}


def _weights_and_counts(target):
    t = np.asarray(target).ravel()
    valid = (t >= 0) & (t < NCLASS)
    N = np.bincount(t[valid].astype(np.int64), minlength=NCLASS).astype(np.float64)
    with np.errstate(over="ignore"):
        w = np.where(N > 0, (1.0 - BETA) / (1.0 - np.power(np.float64(BETA), N)), 0.0)
    return w, N, int(valid.sum())


def _run_fast(logits, target, trace=False, no_ignore=False):
    key = "fast2" if no_ignore else "fast"
    if key not in _CACHE:
        _CACHE[key] = _build_fast2() if no_ignore else _build_fast()
    nc = _CACHE[key]
    lg = np.asarray(logits)
    tg = np.asarray(target)
    in_maps = []
    for i in range(NCORES):
        in_maps.append({
            "logits": np.ascontiguousarray(
                lg[i].reshape(NCLASS, P, COLS)).astype(ml_dtypes.bfloat16),
            "target": np.ascontiguousarray(
                tg[i].reshape(P, COLS)).astype(np.float32),
        })
    return run_bass_kernel_spmd(nc, in_maps, core_ids=list(range(NCORES)), trace=trace)


def _combine_fast(results, w, N, n_valid):
    G1 = 0.0
    G2 = 0.0
    for i in range(NCORES):
        r = results[i]["out"].astype(np.float64).reshape(ACC_COLS)
        G1 += r[:NG].sum()
        G2 += r[NG:].sum()
    # equal weights cancel in the ratio
    return np.float32((G2 - G1) / n_valid)


def _run_exact(logits, target, trace=False):
    if "exact" not in _CACHE:
        _CACHE["exact"] = _build_exact()
    nc = _CACHE["exact"]
    in_maps = []
    for i in range(NCORES):
        in_maps.append({
            "logits": np.ascontiguousarray(
                np.asarray(logits)[i].reshape(NCLASS, P, COLS)),
            "target": np.ascontiguousarray(
                np.asarray(target)[i].reshape(P, COLS)),
        })
    return run_bass_kernel_spmd(nc, in_maps, core_ids=list(range(NCORES)), trace=trace)


def _combine_exact(results, w):
    F0 = 512
    NCH0 = COLS // F0
    A = np.zeros(NCLASS, np.float64)
    B = np.zeros(NCLASS, np.float64)
    N = np.zeros(NCLASS, np.float64)
    for i in range(NCORES):
        r = results[i]["out"].astype(np.float64).reshape(3, NCH0, NCLASS).sum(axis=1)
        A += r[0]
        B += r[1]
        N += r[2]
    num = float((w * (B - A)).sum())
    den = float((w * N).sum())
    return np.float32(num / den)


def kernel(logits, target):
    assert logits.shape == (NCORES, NCLASS, 512, 1024) and logits.dtype == np.float32
    assert target.shape == (NCORES, 512, 1024) and target.dtype == np.int32
    w, N, n_valid = _weights_and_counts(target)
    pos = w[N > 0]
    equal_w = pos.size > 0 and (pos.max() - pos.min()) <= 1e-9 * pos.mean()
    if equal_w:
        no_ignore = n_valid == target.size
        r = _run_fast(logits, target, no_ignore=no_ignore)
        return _combine_fast(r.results, w, N, n_valid)
    r = _run_exact(logits, target)
    return _combine_exact(r.results, w)


# revision 12
# speedup vs baseline: 1.1139x; 1.1139x over previous
"""Class-balanced softmax cross-entropy loss on 8 Trainium2 NeuronCores.

Math: counts N_c over batch; w_c = (1-beta)/(1-beta^N_c) (0 if N_c=0);
loss = -sum w[t](logp[t]) / sum w[t] over valid pixels.

Fast path (used when all class weights are equal, which holds whenever every
class count N_c is large enough that beta^N_c underflows — always true for
this problem's 4.2M uniformly distributed pixels; verified exactly on host
via bincount): the weights cancel in the ratio, so
  loss = (sum_pix lse - sum_pix x[t]) / N_valid
Device computes per core: G1 = sum x[t] (gather via 19 is_equal stt accums
per chunk), G2 = sum_{t>=0} lse (one masked stt per chunk), with
lse = ln(sum_c exp(x_c)) via ACT exp + DVE pairwise-tree adds (f16 TT ops
hit the 2x DVE mode; strided tensor_reduce does not).
Inputs are host-cast: logits -> bf16 (halves HBM traffic; error ~1e-3 rel),
target -> f32.

Exact fallback path (any weight spread): original per-class A/B/N kernel.
"""

import numpy as np
import sys

for _p in ("/opt/trn_rl_repo",):
    if _p not in sys.path:
        sys.path.insert(0, _p)

import ml_dtypes
from concourse import bass, mybir
from concourse.bass_utils import run_bass_kernel_spmd

NCLASS = 19
BETA = 0.999
NCORES = 8
P = 128
COLS = 4096              # 512*1024 / 128
F = 1024                 # free-dim chunk
NCHUNK = COLS // F       # 4

f32 = mybir.dt.float32
f16 = mybir.dt.float16
bf16 = mybir.dt.bfloat16
i32 = mybir.dt.int32
AF = mybir.ActivationFunctionType
ALU = mybir.AluOpType

NG = NCHUNK * NCLASS         # 76 gather columns
ACC_COLS = NG + NCHUNK       # + per-chunk masked-lse columns = 80


def _build_fast():
    nc = bass.Bass()
    logits = nc.declare_dram_parameter("logits", [NCLASS, P, COLS], bf16, isOutput=False)
    target = nc.declare_dram_parameter("target", [P, COLS], f32, isOutput=False)
    out = nc.declare_dram_parameter("out", [1, ACC_COLS], f32, isOutput=True)

    EF = NCLASS * F
    X2 = nc.alloc_sbuf_tensor("X2", [P, 2 * EF], bf16)
    E2 = nc.alloc_sbuf_tensor("E2", [P, 2 * EF], f16)
    T = nc.alloc_sbuf_tensor("T", [P, COLS], f32)
    L2 = nc.alloc_sbuf_tensor("L2", [P, 2 * F], f16)
    junk = nc.alloc_sbuf_tensor("junk", [P, F], f16)
    ABN = nc.alloc_sbuf_tensor("ABN", [P, ACC_COLS], f32)
    ones = nc.alloc_sbuf_tensor("ones", [P, 1], f32)
    res = nc.alloc_sbuf_tensor("res", [1, ACC_COLS], f32)
    ps = nc.alloc_psum_tensor("ps", [1, ACC_COLS], f32)

    with (
        nc.Block() as block,
        nc.semaphore("s_x") as s_x,
        nc.semaphore("s_t") as s_t,
        nc.semaphore("s_exp") as s_exp,
        nc.semaphore("s_tree") as s_tree,
        nc.semaphore("s_log") as s_log,
        nc.semaphore("s_gA") as s_gA,
        nc.semaphore("s_gB") as s_gB,
        nc.semaphore("s_mm") as s_mm,
        nc.semaphore("s_out") as s_out,
        nc.allow_low_precision("f16 tree-sum of exp; error ~0.1% on lse"),
    ):
        @block.sync
        def _(sp):
            sp.dma_start(T[:], target[:, :]).then_inc(s_t, 16)
            for k in range(NCHUNK):
                h = k % 2
                if k >= 2:
                    # X half reused: chunk k-2's gathers (which read X) done
                    sp.wait_ge(s_gA, k - 1)
                sp.dma_start(
                    X2[:, h * EF:(h + 1) * EF].rearrange("p (c f) -> p c f", c=NCLASS),
                    logits[:, :, k * F:(k + 1) * F].rearrange("c p f -> p c f"),
                ).then_inc(s_x, 16)

        @block.scalar
        def _(act):
            for k in range(NCHUNK):
                h = k % 2
                act.wait_ge(s_x, 16 * (k + 1))
                E = E2[:, h * EF:(h + 1) * EF]
                X = X2[:, h * EF:(h + 1) * EF]
                for c in range(NCLASS):
                    ins = act.activation(
                        E[:, c * F:(c + 1) * F], X[:, c * F:(c + 1) * F], AF.Exp)
                    if c == NCLASS - 1:
                        ins.then_inc(s_exp, 1)
                # tree folds sumexp into E[:, 0:F]
                act.wait_ge(s_tree, k + 1)
                if k >= 2:
                    act.wait_ge(s_gB, k - 1)   # L half reused
                act.activation(
                    L2[:, h * F:(h + 1) * F], E[:, 0:F], AF.Ln,
                ).then_inc(s_log, 1)
            # tail: psum -> sbuf -> dram
            act.wait_ge(s_mm, 1)
            act.copy(res[:], ps[:])
            act.dma_start(out[:, :], res[:]).then_inc(s_out, 16)
            act.wait_ge(s_out, 16)

        @block.vector
        def _(dve):
            dve.memset(ABN[:], 0.0)
            dve.memset(ones[:], 1.0)
            dve.wait_ge(s_t, 16)
            for k in range(NCHUNK):
                h = k % 2
                dve.wait_ge(s_exp, k + 1)   # E ready (implies X landed)
                E = E2[:, h * EF:(h + 1) * EF]
                # pairwise tree: 19 -> 16 -> 8 -> 4 -> 2 -> 1 (contiguous f16
                # TT adds run in the DVE 2x mode)
                dve.tensor_tensor(out=E[:, 0:3 * F], in0=E[:, 0:3 * F],
                                  in1=E[:, 16 * F:19 * F], op=ALU.add)
                dve.tensor_tensor(out=E[:, 0:8 * F], in0=E[:, 0:8 * F],
                                  in1=E[:, 8 * F:16 * F], op=ALU.add)
                dve.tensor_tensor(out=E[:, 0:4 * F], in0=E[:, 0:4 * F],
                                  in1=E[:, 4 * F:8 * F], op=ALU.add)
                dve.tensor_tensor(out=E[:, 0:2 * F], in0=E[:, 0:2 * F],
                                  in1=E[:, 2 * F:4 * F], op=ALU.add)
                dve.tensor_tensor(out=E[:, 0:F], in0=E[:, 0:F],
                                  in1=E[:, F:2 * F], op=ALU.add).then_inc(s_tree, 1)
                # gather: G1 partials, one stt per class
                X = X2[:, h * EF:(h + 1) * EF]
                Tk = T[:, k * F:(k + 1) * F]
                for c in range(NCLASS):
                    ins = dve.scalar_tensor_tensor(
                        out=junk[:], in0=Tk, scalar=float(c),
                        in1=X[:, c * F:(c + 1) * F],
                        op0=ALU.is_equal, op1=ALU.mult,
                        accum_out=ABN[:, k * NCLASS + c: k * NCLASS + c + 1])
                    if c == NCLASS - 1:
                        ins.then_inc(s_gA, 1)
                # masked lse sum: G2 partial
                dve.wait_ge(s_log, k + 1)
                dve.scalar_tensor_tensor(
                    out=junk[:], in0=Tk, scalar=-0.5,
                    in1=L2[:, h * F:(h + 1) * F],
                    op0=ALU.is_gt, op1=ALU.mult,
                    accum_out=ABN[:, NG + k: NG + k + 1]).then_inc(s_gB, 1)

        @block.tensor
        def _(pe):
            pe.wait_ge(s_gB, NCHUNK)
            pe.matmul(ps[:], lhsT=ones[:], rhs=ABN[:], start=True, stop=True
                      ).then_inc(s_mm, 1)

    return nc


def _build_fast2():
    """No-ignore fast path: gathers reordered ahead of the exp-dependent
    tree, X DMAs split by class halves for a short pipeline fill, and the
    lse sum folded into the Ln op's accum_out on ACT."""
    nc = bass.Bass()
    logits = nc.declare_dram_parameter("logits", [NCLASS, P, COLS], bf16, isOutput=False)
    target = nc.declare_dram_parameter("target", [P, COLS], f32, isOutput=False)
    out = nc.declare_dram_parameter("out", [1, ACC_COLS], f32, isOutput=True)

    EF = NCLASS * F
    CSPLIT = 10              # classes 0..9 in the first DMA half
    X2 = nc.alloc_sbuf_tensor("X2", [P, 2 * EF], bf16)
    E2 = nc.alloc_sbuf_tensor("E2", [P, 2 * EF], f16)
    T = nc.alloc_sbuf_tensor("T", [P, COLS], f32)
    junk = nc.alloc_sbuf_tensor("junk", [P, F], f16)
    junkA = nc.alloc_sbuf_tensor("junkA", [P, F], f16)
    ABN = nc.alloc_sbuf_tensor("ABN", [P, ACC_COLS], f32)
    ones = nc.alloc_sbuf_tensor("ones", [P, 1], f32)
    res = nc.alloc_sbuf_tensor("res", [1, ACC_COLS], f32)
    ps = nc.alloc_psum_tensor("ps", [1, ACC_COLS], f32)

    with (
        nc.Block() as block,
        nc.semaphore("s_x") as s_x,
        nc.semaphore("s_t") as s_t,
        nc.semaphore("s_exp") as s_exp,
        nc.semaphore("s_tree") as s_tree,
        nc.semaphore("s_gA") as s_gA,
        nc.semaphore("s_lse") as s_lse,
        nc.semaphore("s_mm") as s_mm,
        nc.semaphore("s_out") as s_out,
        nc.allow_low_precision("f16 tree-sum of exp; error ~0.1% on lse"),
    ):
        @block.sync
        def _(sp):
            for k in range(NCHUNK):
                h = k % 2
                if k >= 2:
                    sp.wait_ge(s_gA, k - 1)
                    sp.wait_ge(s_exp, k - 1)
                sp.dma_start(T[:, k * F:(k + 1) * F],
                             target[:, k * F:(k + 1) * F]).then_inc(s_t, 16)
                Xh = X2[:, h * EF:(h + 1) * EF].rearrange("p (c f) -> p c f", c=NCLASS)
                src = logits[:, :, k * F:(k + 1) * F].rearrange("c p f -> p c f")
                sp.dma_start(Xh[:, :CSPLIT], src[:, :CSPLIT]).then_inc(s_x, 16)
                sp.dma_start(Xh[:, CSPLIT:], src[:, CSPLIT:]).then_inc(s_x, 16)

        @block.scalar
        def _(act):
            for k in range(NCHUNK):
                h = k % 2
                E = E2[:, h * EF:(h + 1) * EF]
                X = X2[:, h * EF:(h + 1) * EF]
                groups = [(0, CSPLIT, 32 * k + 16), (CSPLIT, NCLASS, 32 * k + 32)]
                for lo, hi, thr in groups:
                    act.wait_ge(s_x, thr)
                    for c in range(lo, hi):
                        ins = act.activation(
                            E[:, c * F:(c + 1) * F], X[:, c * F:(c + 1) * F], AF.Exp)
                        if c == NCLASS - 1:
                            ins.then_inc(s_exp, 1)
                act.wait_ge(s_tree, k + 1)
                act.activation(
                    junkA[:], E[:, 0:F], AF.Ln,
                    accum_out=ABN[:, NG + k: NG + k + 1]).then_inc(s_lse, 1)
            act.wait_ge(s_mm, NCHUNK + 1)
            act.copy(res[:], ps[:])
            act.dma_start(out[:, :], res[:]).then_inc(s_out, 16)
            act.wait_ge(s_out, 16)

        @block.vector
        def _(dve):
            dve.memset(ABN[:], 0.0)
            dve.memset(ones[:], 1.0)
            for k in range(NCHUNK):
                h = k % 2
                X = X2[:, h * EF:(h + 1) * EF]
                E = E2[:, h * EF:(h + 1) * EF]
                Tk = T[:, k * F:(k + 1) * F]
                dve.wait_ge(s_t, 16 * (k + 1))

                def _tree():
                    dve.wait_ge(s_exp, k + 1)
                    dve.tensor_tensor(out=E[:, 0:3 * F], in0=E[:, 0:3 * F],
                                      in1=E[:, 16 * F:19 * F], op=ALU.add)
                    dve.tensor_tensor(out=E[:, 0:8 * F], in0=E[:, 0:8 * F],
                                      in1=E[:, 8 * F:16 * F], op=ALU.add)
                    dve.tensor_tensor(out=E[:, 0:4 * F], in0=E[:, 0:4 * F],
                                      in1=E[:, 4 * F:8 * F], op=ALU.add)
                    dve.tensor_tensor(out=E[:, 0:2 * F], in0=E[:, 0:2 * F],
                                      in1=E[:, 2 * F:4 * F], op=ALU.add)
                    dve.tensor_tensor(out=E[:, 0:F], in0=E[:, 0:F],
                                      in1=E[:, F:2 * F], op=ALU.add
                                      ).then_inc(s_tree, 1)

                def _gathers():
                    # descending class order opposes ACT's ascending exp
                    # order, minimizing same-region SBUF port contention
                    groups = [(0, CSPLIT, 32 * k + 16),
                              (CSPLIT, NCLASS, 32 * k + 32)]
                    for lo, hi, thr in groups:
                        dve.wait_ge(s_x, thr)
                        for c in reversed(range(lo, hi)):
                            ins = dve.scalar_tensor_tensor(
                                out=junk[:], in0=Tk, scalar=float(c),
                                in1=X[:, c * F:(c + 1) * F],
                                op0=ALU.is_equal, op1=ALU.mult,
                                accum_out=ABN[:, k * NCLASS + c: k * NCLASS + c + 1])
                            if c == lo and hi == NCLASS:
                                ins.then_inc(s_gA, 1)

                _gathers()
                _tree()

        @block.tensor
        def _(pe):
            # per-chunk partition-reduces keep the tail short
            for k in range(NCHUNK):
                pe.wait_ge(s_gA, k + 1)
                pe.matmul(ps[:, k * NCLASS:(k + 1) * NCLASS], lhsT=ones[:],
                          rhs=ABN[:, k * NCLASS:(k + 1) * NCLASS],
                          start=True, stop=True).then_inc(s_mm, 1)
            pe.wait_ge(s_lse, NCHUNK)
            pe.matmul(ps[:, NG:], lhsT=ones[:], rhs=ABN[:, NG:],
                      start=True, stop=True).then_inc(s_mm, 1)

    return nc


def _build_exact():
    """Original per-class A/B/N kernel (correct for any weight pattern)."""
    nc = bass.Bass()
    F0 = 512
    NCH0 = COLS // F0
    SEC = NCH0 * NCLASS
    ACC0 = 3 * SEC
    logits = nc.declare_dram_parameter("logits", [NCLASS, P, COLS], f32, isOutput=False)
    target = nc.declare_dram_parameter("target", [P, COLS], i32, isOutput=False)
    out = nc.declare_dram_parameter("out", [1, ACC0], f32, isOutput=True)

    EF = NCLASS * F0
    X2 = nc.alloc_sbuf_tensor("X2", [P, 2 * EF], f32)
    E2 = nc.alloc_sbuf_tensor("E2", [P, 2 * EF], f32)
    Ti2 = nc.alloc_sbuf_tensor("Ti2", [P, 2 * F0], i32)
    Tf2 = nc.alloc_sbuf_tensor("Tf2", [P, 2 * F0], f32)
    S2 = nc.alloc_sbuf_tensor("S2", [P, 2 * F0], f32)
    L2 = nc.alloc_sbuf_tensor("L2", [P, 2 * F0], f32)
    junk = nc.alloc_sbuf_tensor("junk", [P, F0], f32)
    ABN = nc.alloc_sbuf_tensor("ABN", [P, ACC0], f32)
    ones = nc.alloc_sbuf_tensor("ones", [P, 1], f32)
    res = nc.alloc_sbuf_tensor("res", [1, ACC0], f32)
    ps = nc.alloc_psum_tensor("ps", [1, ACC0], f32)

    with (
        nc.Block() as block,
        nc.semaphore("sem_x") as sem_x,
        nc.semaphore("sem_t") as sem_t,
        nc.semaphore("sem_exp") as sem_exp,
        nc.semaphore("sem_red") as sem_red,
        nc.semaphore("sem_log") as sem_log,
        nc.semaphore("sem_done") as sem_done,
        nc.semaphore("sem_mm") as sem_mm,
        nc.semaphore("sem_out") as sem_out,
    ):
        @block.scalar
        def _(act):
            for k in range(NCH0):
                h = k % 2
                if k >= 2:
                    act.wait_ge(sem_done, k - 1)
                act.dma_start(
                    X2[:, h * EF:(h + 1) * EF].rearrange("p (c f) -> p c f", c=NCLASS),
                    logits[:, :, k * F0:(k + 1) * F0].rearrange("c p f -> p c f"),
                ).then_inc(sem_x, 16)
                act.dma_start(
                    Ti2[:, h * F0:(h + 1) * F0], target[:, k * F0:(k + 1) * F0],
                ).then_inc(sem_t, 16)
                act.wait_ge(sem_x, 16 * (k + 1))
                for c in range(NCLASS):
                    ins = act.activation(
                        E2[:, h * EF + c * F0: h * EF + (c + 1) * F0],
                        X2[:, h * EF + c * F0: h * EF + (c + 1) * F0], AF.Exp)
                    if c == NCLASS - 1:
                        ins.then_inc(sem_exp, 1)
                act.wait_ge(sem_red, k + 1)
                act.activation(
                    L2[:, h * F0:(h + 1) * F0], S2[:, h * F0:(h + 1) * F0], AF.Ln,
                ).then_inc(sem_log, 1)
            act.wait_ge(sem_mm, 1)
            act.copy(res[:], ps[:])
            act.dma_start(out[:, :], res[:]).then_inc(sem_out, 16)
            act.wait_ge(sem_out, 16)

        @block.vector
        def _(dve):
            dve.memset(ABN[:], 0.0)
            dve.memset(ones[:], 1.0)
            for k in range(NCH0):
                h = k % 2
                dve.wait_ge(sem_exp, k + 1)
                dve.tensor_reduce(
                    S2[:, h * F0:(h + 1) * F0],
                    E2[:, h * EF:(h + 1) * EF].rearrange("p (c f) -> p f c", c=NCLASS),
                    axis=mybir.AxisListType.X, op=ALU.add,
                ).then_inc(sem_red, 1)
                dve.wait_ge(sem_t, 16 * (k + 1))
                Ti = Tf2[:, h * F0:(h + 1) * F0]
                dve.tensor_copy(Ti[:], Ti2[:, h * F0:(h + 1) * F0])
                for c in range(NCLASS):
                    dve.scalar_tensor_tensor(
                        out=junk[:], in0=Ti[:], scalar=float(c),
                        in1=X2[:, h * EF + c * F0: h * EF + (c + 1) * F0],
                        op0=ALU.is_equal, op1=ALU.mult,
                        accum_out=ABN[:, 0 * SEC + k * NCLASS + c: 0 * SEC + k * NCLASS + c + 1])
                dve.wait_ge(sem_log, k + 1)
                LSE = L2[:, h * F0:(h + 1) * F0]
                for c in range(NCLASS):
                    dve.scalar_tensor_tensor(
                        out=junk[:], in0=Ti[:], scalar=float(c), in1=LSE[:],
                        op0=ALU.is_equal, op1=ALU.mult,
                        accum_out=ABN[:, 1 * SEC + k * NCLASS + c: 1 * SEC + k * NCLASS + c + 1])
                for c in range(NCLASS):
                    ins = dve.tensor_scalar(
                        out=junk[:], in0=Ti[:], scalar1=float(c), scalar2=None,
                        op0=ALU.is_equal, op1=ALU.add,
                        accum_out=ABN[:, 2 * SEC + k * NCLASS + c: 2 * SEC + k * NCLASS + c + 1])
                    if c == NCLASS - 1:
                        ins.then_inc(sem_done, 1)

        @block.tensor
        def _(pe):
            pe.wait_ge(sem_done, NCH0)
            pe.matmul(ps[:], lhsT=ones[:], rhs=ABN[:], start=True, stop=True).then_inc(sem_mm, 1)

    return nc


_CACHE = {}


def _weights_and_counts(target):
    t = np.asarray(target).ravel()
    valid = (t >= 0) & (t < NCLASS)
    N = np.bincount(t[valid].astype(np.int64), minlength=NCLASS).astype(np.float64)
    with np.errstate(over="ignore"):
        w = np.where(N > 0, (1.0 - BETA) / (1.0 - np.power(np.float64(BETA), N)), 0.0)
    return w, N, int(valid.sum())


def _run_fast(logits, target, trace=False, no_ignore=False):
    key = "fast2" if no_ignore else "fast"
    if key not in _CACHE:
        _CACHE[key] = _build_fast2() if no_ignore else _build_fast()
    nc = _CACHE[key]
    lg = np.asarray(logits)
    tg = np.asarray(target)
    in_maps = []
    for i in range(NCORES):
        in_maps.append({
            "logits": np.ascontiguousarray(
                lg[i].reshape(NCLASS, P, COLS)).astype(ml_dtypes.bfloat16),
            "target": np.ascontiguousarray(
                tg[i].reshape(P, COLS)).astype(np.float32),
        })
    return run_bass_kernel_spmd(nc, in_maps, core_ids=list(range(NCORES)), trace=trace)


def _combine_fast(results, w, N, n_valid):
    G1 = 0.0
    G2 = 0.0
    for i in range(NCORES):
        r = results[i]["out"].astype(np.float64).reshape(ACC_COLS)
        G1 += r[:NG].sum()
        G2 += r[NG:].sum()
    # equal weights cancel in the ratio
    return np.float32((G2 - G1) / n_valid)


def _run_exact(logits, target, trace=False):
    if "exact" not in _CACHE:
        _CACHE["exact"] = _build_exact()
    nc = _CACHE["exact"]
    in_maps = []
    for i in range(NCORES):
        in_maps.append({
            "logits": np.ascontiguousarray(
                np.asarray(logits)[i].reshape(NCLASS, P, COLS)),
            "target": np.ascontiguousarray(
                np.asarray(target)[i].reshape(P, COLS)),
        })
    return run_bass_kernel_spmd(nc, in_maps, core_ids=list(range(NCORES)), trace=trace)


def _combine_exact(results, w):
    F0 = 512
    NCH0 = COLS // F0
    A = np.zeros(NCLASS, np.float64)
    B = np.zeros(NCLASS, np.float64)
    N = np.zeros(NCLASS, np.float64)
    for i in range(NCORES):
        r = results[i]["out"].astype(np.float64).reshape(3, NCH0, NCLASS).sum(axis=1)
        A += r[0]
        B += r[1]
        N += r[2]
    num = float((w * (B - A)).sum())
    den = float((w * N).sum())
    return np.float32(num / den)


def kernel(logits, target):
    assert logits.shape == (NCORES, NCLASS, 512, 1024) and logits.dtype == np.float32
    assert target.shape == (NCORES, 512, 1024) and target.dtype == np.int32
    w, N, n_valid = _weights_and_counts(target)
    pos = w[N > 0]
    equal_w = pos.size > 0 and (pos.max() - pos.min()) <= 1e-9 * pos.mean()
    if equal_w:
        no_ignore = n_valid == target.size
        r = _run_fast(logits, target, no_ignore=no_ignore)
        return _combine_fast(r.results, w, N, n_valid)
    r = _run_exact(logits, target)
    return _combine_exact(r.results, w)
